# revision 10
# baseline (speedup 1.0000x reference)
"""GraphGym GeneralConv (GCN-style, add-aggr, symmetric norm) on 8 Trainium2
NeuronCores via Bass/Tile.

Math (matches the reference exactly, up to fp reassociation):
    deg[i]  = 1 + #{e : row[e] == i}            (self-loop appended)
    dis     = deg ** -0.5
    h       = x @ W
    out[i]  = dis[i] * sum_{e : col[e] == i} dis[row[e]] * h[row[e]]
              (self-loop (i, i) included as an ordinary edge)

Distribution: destination-node sharding.  Core k owns dest nodes
[k*SHARD, (k+1)*SHARD); every edge is routed to the core owning its dest.
Every core redundantly computes the full h = x @ W (x is replicated), writes
it to a DRAM scratch `h_perm` in a permuted row layout, then gathers its
edges' source rows with the SWDGE dma_gather instruction (edge-major output),
scatter-adds them into per-dest-block PSUM accumulators with
selection-matrix matmuls on the PE, scales by dis[dest], and writes its
shard.  The host does integer-only preprocessing: edge bucketing by
(core, dest-block, source-chunk), degree histogram, and index packing.

The slot layout is input-independent (fixed run length L_RUN per
(dest-block, chunk) bucket), so the Bass program is compiled once and cached.
Bucket overflow (probability ~0 for random graphs) is corrected on the host.
"""

import math
import os

import numpy as np

# ----------------------------------------------------------------------------
# configuration
# ----------------------------------------------------------------------------

N_NODES = 100000
DIM = 64
N_CORES = 8

P = 128  # partitions


class Cfg:
    def __init__(self, n_nodes, dim, n_cores, slice_tiles, chunk_slices,
                 tiles_per_run, blocks_per_group):
        self.N = n_nodes
        self.DIM = dim
        self.NC = n_cores
        assert n_nodes % n_cores == 0
        self.SHARD = n_nodes // n_cores          # dest nodes per core
        self.NBLK = math.ceil(self.SHARD / P)    # dest blocks per core
        self.SLICE = P * slice_tiles             # h-permutation slice (nodes)
        self.NS = math.ceil(n_nodes / self.SLICE)
        # tiles per slice (last slice may be partial)
        self.J = [min(slice_tiles, math.ceil((n_nodes - s * self.SLICE) / P))
                  for s in range(self.NS)]
        self.row_base = np.concatenate([[0], np.cumsum([P * j for j in self.J])])
        self.H_ROWS = int(self.row_base[-1])
        self.CH_SL = chunk_slices                # slices per gather chunk
        self.NCH = math.ceil(self.NS / chunk_slices)
        # chunk row windows in permuted space
        self.crow = [int(self.row_base[min(c * chunk_slices, self.NS)])
                     for c in range(self.NCH + 1)]
        for c in range(self.NCH):
            assert self.crow[c + 1] - self.crow[c] <= 32767, "chunk window too big"
        self.TPR = tiles_per_run                 # 128-slot tiles per run
        self.L_RUN = P * tiles_per_run           # slots per (block, chunk) run
        self.NBG = blocks_per_group              # dest blocks per psum group
        assert self.NBLK % blocks_per_group == 0
        self.NGRP = self.NBLK // blocks_per_group
        self.NRUNS = self.NGRP * self.NCH * self.NBG   # runs per core
        self.TOT = self.NRUNS * self.L_RUN             # slots per core
        self.NTILES = self.TOT // P
        self.CALL_SLOTS = self.NBG * self.L_RUN        # slots per gather call
        self.NCALLS = self.NGRP * self.NCH
        self.IDXW = self.TOT // 16
        assert self.CALL_SLOTS % 16 == 0


CFG = Cfg(N_NODES, DIM, N_CORES, slice_tiles=100, chunk_slices=2,
          tiles_per_run=5, blocks_per_group=7)


def rho(cfg, n):
    """node id -> permuted h_perm row (vectorized)."""
    s = n // cfg.SLICE
    m = n - s * cfg.SLICE
    J = np.asarray(cfg.J)[s]
    return cfg.row_base[s] + (m % P) * J + m // P


# ----------------------------------------------------------------------------
# host preprocessing
# ----------------------------------------------------------------------------

def host_prep(cfg, x, weight, edge_index):
    x = np.asarray(x, dtype=np.float32)
    weight = np.asarray(weight, dtype=np.float32)
    ei = np.asarray(edge_index)
    row = ei[0].astype(np.int64)
    col = ei[1].astype(np.int64)

    # deg counts outgoing (row) edges plus the implicit self-loop
    deg = (np.bincount(row, minlength=cfg.N) + 1).astype(np.float32)
    # self-loop messages are NOT routed through the edge pipeline; they are
    # added per dest block on-device via an indirect row gather + diagonal
    # matmul (see build_program), so `row`/`col` stay real edges only.

    k = col // cfg.SHARD
    blk = (col % cfg.SHARD) // P
    col_local = (col % cfg.SHARD) % P
    g = blk // cfg.NBG
    b_ = blk % cfg.NBG
    s = row // cfg.SLICE
    c = s // cfg.CH_SL
    prow = rho(cfg, row)
    idxrel = prow - np.asarray(cfg.crow)[c]

    run_in_core = (g * cfg.NCH + c) * cfg.NBG + b_
    key = k * cfg.NRUNS + run_in_core

    order = np.argsort(key, kind="stable")
    key_s = key[order]
    counts = np.bincount(key_s, minlength=cfg.NC * cfg.NRUNS)
    starts = np.concatenate([[0], np.cumsum(counts)])
    pos = np.arange(key_s.size) - starts[key_s]

    ok = pos < cfg.L_RUN
    slot = run_in_core[order] * cfg.L_RUN + pos   # slot within core
    kk = k[order]

    idx_flat = np.zeros((cfg.NC, cfg.TOT), dtype=np.int16)
    colv = np.full((cfg.NC, cfg.TOT), -1.0, dtype=np.float32)
    degrow = np.ones((cfg.NC, cfg.TOT), dtype=np.float32)

    o = order[ok]
    idx_flat[kk[ok], slot[ok]] = idxrel[o].astype(np.int16)
    colv[kk[ok], slot[ok]] = col_local[o].astype(np.float32)
    degrow[kk[ok], slot[ok]] = deg[row[o]]

    # overflow edges -> host correction (expected empty)
    ov = order[~ok]

    # per-call 16-partition wrapping of indices, replicated to 128 partitions
    idxw = idx_flat.reshape(cfg.NC, cfg.NCALLS, cfg.CALL_SLOTS // 16, 16)
    idxw = idxw.transpose(0, 3, 1, 2).reshape(cfg.NC, 16, cfg.IDXW)
    idxw = np.ascontiguousarray(np.tile(idxw, (1, 8, 1)))

    colv_p = np.ascontiguousarray(
        colv.reshape(cfg.NC, cfg.NTILES, P).transpose(0, 2, 1))
    degrow_p = np.ascontiguousarray(
        degrow.reshape(cfg.NC, cfg.NTILES, P).transpose(0, 2, 1))

    # dest-side degrees [NC, 128, NBLK]
    degdest = np.ones((cfg.NC, cfg.NBLK * P), dtype=np.float32)
    ids = np.arange(cfg.SHARD)
    for core in range(cfg.NC):
        degdest[core, :cfg.SHARD] = deg[core * cfg.SHARD + ids]
    degdest = np.ascontiguousarray(
        degdest.reshape(cfg.NC, cfg.NBLK, P).transpose(0, 2, 1))

    # per-block self rows: permuted h row of each dest node [NC, 128, NBLK]
    selfidx = np.zeros((cfg.NC, cfg.NBLK * P), dtype=np.int32)
    for core in range(cfg.NC):
        selfidx[core, :cfg.SHARD] = rho(cfg, core * cfg.SHARD + ids)
    selfidx = np.ascontiguousarray(
        selfidx.reshape(cfg.NC, cfg.NBLK, P).transpose(0, 2, 1))

    xt = np.ascontiguousarray(x.T)
    iota = np.broadcast_to(np.arange(P, dtype=np.float32), (P, P)).copy()
    partidx = np.arange(P, dtype=np.float32).reshape(P, 1).copy()

    in_maps = []
    for core in range(cfg.NC):
        in_maps.append({
            "xt": xt,
            "w": weight,
            "iota": iota,
            "partidx": partidx,
            "colv": colv_p[core],
            "degrow": degrow_p[core],
            "degdest": degdest[core],
            "selfidx": selfidx[core],
            "idx": idxw[core],
        })

    # host correction for overflowed edges
    corr = None
    if ov.size:
        r, cdst = row[ov], col[ov]
        hsrc = x[r] @ weight
        m = hsrc * (deg[r] ** -0.5 * deg[cdst] ** -0.5)[:, None]
        corr = np.zeros((cfg.N, cfg.DIM), dtype=np.float32)
        np.add.at(corr, cdst, m)
    return in_maps, corr


def unshard(cfg, outs, corr):
    out = np.empty((cfg.N, cfg.DIM), dtype=np.float32)
    for core in range(cfg.NC):
        o = outs[core]["outp"].reshape(P, cfg.NBLK, cfg.DIM)
        o = o.transpose(1, 0, 2).reshape(cfg.NBLK * P, cfg.DIM)
        out[core * cfg.SHARD:(core + 1) * cfg.SHARD] = o[:cfg.SHARD]
    if corr is not None:
        out += corr
    return out


# ----------------------------------------------------------------------------
# device program
# ----------------------------------------------------------------------------

_PROG_CACHE = {}


def build_program(cfg):
    import concourse.bass as bass
    import concourse.tile as tile
    from concourse import bacc, mybir

    f32 = mybir.dt.float32
    nc = bacc.Bacc("TRN2", target_bir_lowering=False, debug=False,
                   num_devices=cfg.NC)

    xt = nc.dram_tensor("xt", [cfg.DIM, cfg.N], f32, kind="ExternalInput")
    w = nc.dram_tensor("w", [cfg.DIM, cfg.DIM], f32, kind="ExternalInput")
    iota = nc.dram_tensor("iota", [P, P], f32, kind="ExternalInput")
    partidx = nc.dram_tensor("partidx", [P, 1], f32, kind="ExternalInput")
    selfidx = nc.dram_tensor("selfidx", [P, cfg.NBLK], mybir.dt.int32,
                             kind="ExternalInput")
    colv = nc.dram_tensor("colv", [P, cfg.NTILES], f32, kind="ExternalInput")
    degrow = nc.dram_tensor("degrow", [P, cfg.NTILES], f32,
                            kind="ExternalInput")
    degdest = nc.dram_tensor("degdest", [P, cfg.NBLK], f32,
                             kind="ExternalInput")
    idx = nc.dram_tensor("idx", [P, cfg.IDXW], mybir.dt.int16,
                         kind="ExternalInput")
    outp = nc.dram_tensor("outp", [P, cfg.NBLK * cfg.DIM], f32,
                          kind="ExternalOutput")
    h_perm = nc.dram_tensor("h_perm", [cfg.H_ROWS, cfg.DIM], f32)

    PSB = 8  # h tiles batched per psum bank

    with tile.TileContext(nc) as tc:
        # ---------------- phase 1: h = x @ W, permuted layout ----------------
        with tc.tile_pool(name="p1s", bufs=2) as sp, \
             tc.tile_pool(name="p1c", bufs=1) as cp, \
             tc.tile_pool(name="p1p", bufs=4, space="PSUM") as pp:
            w_sb = cp.tile([cfg.DIM, cfg.DIM], f32)
            nc.sync.dma_start(out=w_sb[:], in_=w[:])
            for s in range(cfg.NS):
                J = cfg.J[s]
                n0 = s * cfg.SLICE
                nn = min(cfg.SLICE, cfg.N - n0)
                xs = sp.tile([cfg.DIM, cfg.SLICE], f32, tag="xs")
                nc.sync.dma_start(out=xs[:, :nn], in_=xt[:, n0:n0 + nn])
                if nn < P * J:
                    # pad the partial tail tile so every psum row is written
                    nc.vector.memset(xs[:, nn:P * J], 0)
                hs = sp.tile([P, cfg.J[0] * cfg.DIM], f32, tag="hs")
                for m in range(math.ceil(J / PSB)):
                    j0 = m * PSB
                    jn = min(PSB, J - j0)
                    ps = pp.tile([P, PSB * cfg.DIM], f32)
                    for j8 in range(jn):
                        j = j0 + j8
                        nc.tensor.matmul(
                            out=ps[:, j8 * cfg.DIM:(j8 + 1) * cfg.DIM],
                            lhsT=xs[:, j * P:(j + 1) * P],
                            rhs=w_sb[:],
                            start=True, stop=True)
                    nc.vector.tensor_copy(
                        out=hs[:, j0 * cfg.DIM:(j0 + jn) * cfg.DIM],
                        in_=ps[:, :jn * cfg.DIM])
                dst = h_perm[cfg.row_base[s]:cfg.row_base[s] + P * J, :]
                dst = dst.rearrange("(p j) d -> p (j d)", p=P)
                nc.sync.dma_start(out=dst, in_=hs[:, :J * cfg.DIM])

        # ---------------- phase 2: gather + PE scatter-add ----------------
        with tc.tile_pool(name="p2c", bufs=1) as cp, \
             tc.tile_pool(name="p2g", bufs=3) as gp, \
             tc.tile_pool(name="p2sel", bufs=6) as selp, \
             tc.tile_pool(name="p2p", bufs=2, space="PSUM") as pp:
            iota_sb = cp.tile([P, P], f32)
            nc.sync.dma_start(out=iota_sb[:], in_=iota[:])
            pidx_sb = cp.tile([P, 1], f32)
            nc.sync.dma_start(out=pidx_sb[:], in_=partidx[:])
            colv_sb = cp.tile([P, cfg.NTILES], f32)
            nc.sync.dma_start(out=colv_sb[:], in_=colv[:])
            selv_sb = cp.tile([P, cfg.NTILES], f32)
            nc.sync.dma_start(out=selv_sb[:], in_=degrow[:])
            # dis[row] = 1/sqrt(deg[row])
            nc.scalar.sqrt(out=selv_sb[:], in_=selv_sb[:])
            nc.vector.reciprocal(out=selv_sb[:], in_=selv_sb[:])
            disd_sb = cp.tile([P, cfg.NBLK], f32)
            nc.sync.dma_start(out=disd_sb[:], in_=degdest[:])
            nc.scalar.sqrt(out=disd_sb[:], in_=disd_sb[:])
            nc.vector.reciprocal(out=disd_sb[:], in_=disd_sb[:])
            sidx_sb = cp.tile([P, cfg.NBLK], mybir.dt.int32)
            nc.sync.dma_start(out=sidx_sb[:], in_=selfidx[:])
            idx_sb = cp.tile([P, cfg.IDXW], mybir.dt.int16)
            nc.sync.dma_start(out=idx_sb[:], in_=idx[:])
            out_sb = cp.tile([P, cfg.NBLK * cfg.DIM], f32)

            CW = cfg.CALL_SLOTS // 16  # idx columns per call
            CT = cfg.CALL_SLOTS // P   # slot tiles per call
            for g in range(cfg.NGRP):
                ps = pp.tile([P, cfg.NBG * cfg.DIM], f32)
                for c in range(cfg.NCH):
                    call = g * cfg.NCH + c
                    gbuf = gp.tile([P, CT, cfg.DIM], f32)
                    nc.gpsimd.dma_gather(
                        out_ap=gbuf[:],
                        in_ap=h_perm[cfg.crow[c]:cfg.crow[c + 1], :],
                        idxs_ap=idx_sb[:, call * CW:(call + 1) * CW],
                        num_idxs=cfg.CALL_SLOTS,
                        num_idxs_reg=cfg.CALL_SLOTS,
                        elem_size=cfg.DIM,
                        single_packet=False,
                    )
                    for b_ in range(cfg.NBG):
                        for t in range(cfg.TPR):
                            T = call * CT + b_ * cfg.TPR + t
                            sel = selp.tile([P, P], f32)
                            nc.vector.tensor_scalar(
                                out=sel[:], in0=iota_sb[:],
                                scalar1=colv_sb[:, T:T + 1],
                                scalar2=selv_sb[:, T:T + 1],
                                op0=mybir.AluOpType.is_equal,
                                op1=mybir.AluOpType.mult)
                            nc.tensor.matmul(
                                out=ps[:, b_ * cfg.DIM:(b_ + 1) * cfg.DIM],
                                lhsT=sel[:],
                                rhs=gbuf[:, b_ * cfg.TPR + t, :],
                                start=(b_ == 0 and c == 0 and t == 0),
                                stop=False, skip_group_check=True)
                for b_ in range(cfg.NBG):
                    b = g * cfg.NBG + b_
                    # self-loop: psum[:, b_] += diag(dis[dest]) @ h[dest]
                    hself = selp.tile([P, cfg.DIM], f32, tag="hself")
                    nc.gpsimd.indirect_dma_start(
                        out=hself[:], out_offset=None,
                        in_=h_perm[:],
                        in_offset=bass.IndirectOffsetOnAxis(
                            ap=sidx_sb[:, b:b + 1], axis=0))
                    diag = selp.tile([P, P], f32, tag="diag")
                    nc.vector.tensor_scalar(
                        out=diag[:], in0=iota_sb[:],
                        scalar1=pidx_sb[:, 0:1],
                        scalar2=disd_sb[:, b:b + 1],
                        op0=mybir.AluOpType.is_equal,
                        op1=mybir.AluOpType.mult)
                    nc.tensor.matmul(
                        out=ps[:, b_ * cfg.DIM:(b_ + 1) * cfg.DIM],
                        lhsT=diag[:], rhs=hself[:],
                        start=False, stop=True, skip_group_check=True)
                    nc.vector.tensor_scalar_mul(
                        out_sb[:, b * cfg.DIM:(b + 1) * cfg.DIM],
                        ps[:, b_ * cfg.DIM:(b_ + 1) * cfg.DIM],
                        disd_sb[:, b:b + 1])
            nc.sync.dma_start(out=outp[:], in_=out_sb[:])

    nc.compile()
    return nc


def get_program(cfg):
    key = (cfg.N, cfg.DIM, cfg.NC, cfg.SLICE, cfg.CH_SL, cfg.TPR, cfg.NBG)
    if key not in _PROG_CACHE:
        _PROG_CACHE[key] = build_program(cfg)
    return _PROG_CACHE[key]


# ----------------------------------------------------------------------------
# entry point
# ----------------------------------------------------------------------------

def kernel(x, weight, edge_index):
    from concourse.bass_utils import run_bass_kernel_spmd

    cfg = CFG
    in_maps, corr = host_prep(cfg, x, weight, edge_index)
    nc = get_program(cfg)
    res = run_bass_kernel_spmd(nc, in_maps, list(range(cfg.NC)))
    return unshard(cfg, res.results, corr)
